# revision 3
# baseline (speedup 1.0000x reference)
"""NT-Xent (SimCLR) contrastive loss on 8 Trainium2 NeuronCores.

Strategy (fully SPMD, no collectives):
  z = normalize(concat(emb_i, emb_j))  # [8192, 512]
  Each core c handles a 1024-row block of z. Inputs are pre-rotated on the
  host (np.roll by -c*1024 rows) so every core runs the identical program on
  rows 0..1023 of its own rotated copy: positive pair of rotated row i is
  rotated row (i + 4096) % 8192 for every core.

  Per core (fp8 pipeline):
    - load emb row-PAIRS per partition ([pair, parity, d]); fp32 norms via
      fused DVE square+reduce; rinv*16 computed as exp(-0.5*ln(s) + ln 16)
    - DVE quantize z*16 -> fp8e4, writing the two parities byte-interleaved
      so the staged DRAM image S[pair, 2d+parity] is contiguous
    - u16-view DMA-xbar transposes of S build plane-separated fp8
      zT [128, 4(k-plane), 8192(rows)] in SBUF
    - sim row-block via fp8 DoubleRow matmuls (256-deep contraction per
      instruction): psum = 256*sim in [128, 4x512] 4-bank PSUM tiles
    - one ACT exp(psum * 2/256) over 2048 elems with free-dim accumulation
      per tile -> row denominators (exp matrix never stored)
    - self-dot and positive-pair dot extracted from the PSUM diagonal via
      DVE identity-mask multiply+reduce (no separate dot passes)
    - loss_row = ln(denom - exp(2*selfdot)) - 2*posdot
  Host: gather 8x1024 row losses, mean.
"""

import math

import numpy as np

import concourse.bacc as bacc
import concourse.tile as tile
from concourse import mybir
from concourse.bass_utils import run_bass_kernel_spmd

N_CORES = 8
D = 512
ROWS = 8192
BLK = ROWS // N_CORES  # 1024
P = 128
BLK_CHUNKS = BLK // P  # 8
NT = 512  # one PSUM bank of fp32
KD = D // P  # 4 contraction planes
SCALE = 16.0  # fp8 quantization scale; psum = SCALE^2 * sim
PSUM_SCALE = SCALE * SCALE

f32 = mybir.dt.float32
bf16 = mybir.dt.bfloat16
fp8 = mybir.dt.float8e4
u16 = mybir.dt.uint16
i32 = mybir.dt.int32

_ACT_PATCHED = False


def _patch_act_tables():
    """Make Exp and Ln resolve only to natural_log_exp_and_others so the
    whole kernel uses a single activation-table set (one ~2.7us load instead
    of one per Ln<->Exp alternation). Preserves dict order so the emitted
    act_func_set_id indices stay aligned with act_info.json."""
    global _ACT_PATCHED
    if _ACT_PATCHED:
        return
    import concourse.hw_specs as hw_specs

    Act = mybir.ActivationFunctionType
    orig = hw_specs.get_activation_tables("gen3")
    patched = {}
    for name, funcs in orig.items():
        fs = set(funcs)
        if name != "natural_log_exp_and_others":
            fs.discard(Act.Exp)
            fs.discard(Act.Ln)
        patched[name] = fs
    bacc.get_activation_tables = lambda arch: patched
    _ACT_PATCHED = True


def _build(loop_k: int = 1):
    _patch_act_tables()
    nc = bacc.Bacc("TRN2", target_bir_lowering=False)
    emb = nc.dram_tensor("emb", [ROWS, D], f32, kind="ExternalInput")
    loss = nc.dram_tensor("loss", [P, BLK_CHUNKS], f32, kind="ExternalOutput")

    with tile.TileContext(nc) as tc:
        with (
            tc.tile_pool(name="persist", bufs=1) as persist,
            tc.tile_pool(name="loads", bufs=4) as loads,
            tc.tile_pool(name="zqs", bufs=2) as zqs,
            tc.tile_pool(name="scratch", bufs=3) as scratch,
            tc.tile_pool(name="small", bufs=2) as small,
            tc.tile_pool(name="dram", bufs=1, space="DRAM") as dram,
            tc.tile_pool(name="psum", bufs=2, space="PSUM") as psum_pool,
        ):
            import contextlib

            loop_ctx = (
                tc.For_i(0, loop_k, 1) if loop_k > 1 else contextlib.nullcontext()
            )
            with loop_ctx:
                _body(nc, tc, persist, loads, zqs, scratch, small, dram, psum_pool, emb, loss)

    nc.compile()
    return nc


def _body(nc, tc, persist, loads, zqs, scratch, small, dram, psum_pool, emb, loss):
    Alu = mybir.AluOpType
    Act = mybir.ActivationFunctionType

    # persistent tensors
    zT16 = persist.tile([P, KD, ROWS // 2], u16, tag="zT16")  # 32 KiB/part
    acc = [
        persist.tile([P, 4], f32, tag=f"acc{m}", name=f"acc{m}")
        for m in range(BLK_CHUNKS)
    ]
    selfd = persist.tile([P, BLK_CHUNKS], f32, tag="selfd")
    posd = persist.tile([P, BLK_CHUNKS], f32, tag="posd")
    ident = persist.tile([P, P], bf16, tag="ident")
    S = dram.tile([ROWS // 2, 2 * D], fp8, tag="S", name="S")  # staged z, pair rows

    # identity mask for PSUM diagonal extraction: 1.0 at [p, p]
    io = small.tile([P, P], i32, tag="io")
    nc.gpsimd.iota(io, pattern=[[1, P]], base=0, channel_multiplier=-1)
    nc.vector.tensor_scalar(
        out=ident, in0=io, scalar1=0, scalar2=None, op0=Alu.is_equal
    )

    # octant 0 holds the block rows; octant 4 the positive pairs. Process
    # those first so the first matmul group (n-tiles 0,1,8,9) starts early.
    octant_order = [0, 4, 1, 2, 3, 5, 6, 7]
    S16 = S[:, :].bitcast(u16)  # [4096, 512] u16 view of staged fp8 pairs
    for oct_ in octant_order:
        sq = small.tile([P, 8], f32, tag="sq")
        ets = []
        for half in range(2):
            et = loads.tile([P, 2, 2, D], f32, tag="et")
            r0 = oct_ * BLK + half * 512
            src = emb[r0 : r0 + 512, :].rearrange(
                "(cg p two) d -> p cg two d", p=P, two=2
            )
            nc.sync.dma_start(out=et, in_=src)
            ets.append(et)
        for half in range(2):
            for cg in range(2):
                for par in range(2):
                    c = half * 4 + cg * 2 + par
                    tt = scratch.tile([P, D], bf16, tag="ttout")
                    # scalar pre-scales the squares by 1/SCALE^2 so that
                    # exp(-0.5*ln(.)) directly yields SCALE/||e|| (no bias
                    # needed -- float activation bias lacks a const AP).
                    nc.vector.scalar_tensor_tensor(
                        out=tt,
                        in0=ets[half][:, cg, par, :],
                        scalar=1.0 / PSUM_SCALE,
                        in1=ets[half][:, cg, par, :],
                        op0=Alu.mult,
                        op1=Alu.mult,
                        accum_out=sq[:, c : c + 1],
                    )
        # SCALE/sqrt(s) = exp(-0.5*ln(s/SCALE^2)) -- single ACT table set
        lnv = small.tile([P, 8], f32, tag="lnv")
        nc.scalar.activation(out=lnv, in_=sq, func=Act.Ln)
        rinv = small.tile([P, 8], f32, tag="rinv")
        nc.scalar.activation(out=rinv, in_=lnv, func=Act.Exp, scale=-0.5)

        zq = zqs.tile([P, 4, 2 * D], fp8, tag="zq")
        for half in range(2):
            for cg in range(2):
                g = half * 2 + cg
                zq_pairs = zq[:, g, :].rearrange("p (d two) -> p two d", two=2)
                for par in range(2):
                    c = half * 4 + cg * 2 + par
                    nc.vector.tensor_scalar_mul(
                        out=zq_pairs[:, par, :],
                        in0=ets[half][:, cg, par, :],
                        scalar1=rinv[:, c : c + 1],
                    )
        # stage octant (512 KiB, contiguous) to DRAM
        dst = S[oct_ * 512 : (oct_ + 1) * 512, :].rearrange("(g p) b -> p g b", p=P)
        nc.scalar.dma_start(out=dst, in_=zq)
        # u16 xbar transposes -> plane-separated fp8 zT columns for this octant
        for t in range(KD):
            nc.sync.dma_start_transpose(
                out=zT16[:, t, oct_ * 512 : (oct_ + 1) * 512],
                in_=S16[oct_ * 512 : (oct_ + 1) * 512, t * P : (t + 1) * P],
            )

    # main loop: 4 n-chunks share a 4-bank PSUM tile; DoubleRow fp8 matmuls
    # contract 256 rows per instruction (plane pairs), h-inner so one
    # stationary operand serves 4 consecutive matmuls.
    zT8 = zT16[:, :, :].bitcast(fp8)  # [128, KD, 8192]
    n_groups = [[0, 1, 8, 9], [2, 3, 4, 5], [6, 7, 10, 11], [12, 13, 14, 15]]
    for ng, group in enumerate(n_groups):
        gw = len(group)
        for m in range(BLK_CHUNKS):
            pst = psum_pool.tile([P, gw, NT], f32, tag="ps", bufs=2)
            for h in range(KD // 2):
                for li, n in enumerate(group):
                    nc.tensor.matmul(
                        pst[:, li, :],
                        zT8[:, 2 * h : 2 * h + 2, m * P : (m + 1) * P],
                        zT8[:, 2 * h : 2 * h + 2, n * NT : (n + 1) * NT],
                        start=(h == 0),
                        stop=(h == KD // 2 - 1),
                        perf_mode=mybir.MatmulPerfMode.DoubleRow,
                    )
            if ng == 0:
                # diagonal extraction: self (n-tiles 0,1) and positive pair
                # (n-tiles 8,9) dots for rows m*128+p, straight from PSUM.
                off = 128 * (m % 4)
                for li, dst_t in ((m // 4, selfd), (2 + m // 4, posd)):
                    dd = scratch.tile([P, P], bf16, tag="ddum")
                    nc.vector.scalar_tensor_tensor(
                        out=dd,
                        in0=pst[:, li, off : off + P],
                        scalar=1.0,
                        in1=ident,
                        op0=Alu.mult,
                        op1=Alu.mult,
                        accum_out=dst_t[:, m : m + 1],
                    )
            ex = scratch.tile([P, gw, NT], bf16, tag="exout")
            nc.scalar.activation(
                out=ex,
                in_=pst,
                func=Act.Exp,
                scale=2.0 / PSUM_SCALE,
                accum_out=acc[m][:, ng : ng + 1],
            )

    # finale: loss_row = ln(denom - exp(2*selfdot)) - 2*posdot
    dsum = persist.tile([P, BLK_CHUNKS], f32, tag="dsum")
    for m in range(BLK_CHUNKS):
        nc.vector.reduce_sum(
            out=dsum[:, m : m + 1], in_=acc[m], axis=mybir.AxisListType.X
        )
    sexp = small.tile([P, BLK_CHUNKS], f32, tag="sexp")
    nc.scalar.activation(out=sexp, in_=selfd, func=Act.Exp, scale=2.0 / PSUM_SCALE)
    dx = small.tile([P, BLK_CHUNKS], f32, tag="dx")
    nc.vector.tensor_sub(dx, dsum, sexp)
    ld = small.tile([P, BLK_CHUNKS], f32, tag="ld")
    nc.scalar.activation(out=ld, in_=dx, func=Act.Ln)
    lossv = small.tile([P, BLK_CHUNKS], f32, tag="lossv")
    nc.vector.scalar_tensor_tensor(
        out=lossv,
        in0=posd,
        scalar=-2.0 / PSUM_SCALE,
        in1=ld,
        op0=Alu.mult,
        op1=Alu.add,
    )
    nc.sync.dma_start(out=loss[:, :], in_=lossv)


_NC_CACHE = []


def _get_nc():
    if not _NC_CACHE:
        _NC_CACHE.append(_build())
    return _NC_CACHE[0]


def make_in_maps(emb_i: np.ndarray, emb_j: np.ndarray):
    emb_all = np.concatenate(
        [np.asarray(emb_i, np.float32), np.asarray(emb_j, np.float32)], axis=0
    )
    return [
        {"emb": np.ascontiguousarray(np.roll(emb_all, -c * BLK, axis=0))}
        for c in range(N_CORES)
    ]


def assemble(results) -> np.ndarray:
    rows = []
    for c in range(N_CORES):
        out = results[c]["loss"]  # [128, 8]; out[p, m] = loss of block row m*128+p
        rows.append(out.T.reshape(-1))
    all_rows = np.concatenate(rows)  # original row order
    return np.float32(all_rows.astype(np.float64).mean())


def kernel(emb_i: np.ndarray, emb_j: np.ndarray) -> np.ndarray:
    nc = _get_nc()
    res = run_bass_kernel_spmd(
        nc, make_in_maps(emb_i, emb_j), core_ids=list(range(N_CORES))
    )
    return assemble(res.results)


if __name__ == "__main__":
    rng = np.random.default_rng(0)
    ei = rng.standard_normal((4096, D)).astype(np.float32)
    ej = rng.standard_normal((4096, D)).astype(np.float32)
    print(kernel(ei, ej))
